# revision 35
# baseline (speedup 1.0000x reference)
"""Trainium2 Bass kernel for sliding-window causal attention (CausalAttention).

Computation (per reference):
    qkv = x @ W_qkv + b_qkv            # [BF, T, 3C]
    split into per-head q, k, v (head dim 64)
    scores = (q @ k.T) / sqrt(hd) + band_bias(pe)   # band: 0 <= i-j <= 31
    out = softmax(scores) @ v          # [BF, T, C]

Sharding: data-parallel over BF across 8 cores (8 bf rows per core).
Device strategy (per core, bf_l=8, tok=2048):
  - host supplies xT = x.T [512, 2048] (fp32r) plus W slices pre-permuted so
    q/k come out transposed ([channel, token], head-pair per partition tile)
    and v natural ([token, channel]).  q lands in parity-split zero-padded
    tiles so every score matmul contracts over the full 128 partitions
    (mixed 64-row tile_position streams fault the PE).
  - scores are computed transposed (ST[key, query]) per query-block window,
    so softmax normalization lands on the partition (query) axis after the
    attention*V matmul, where it is a cheap per-partition scale.
  - four heads share a PSUM bank per score block: band-bias add (matrices
    host-expanded from pe) and exp run as one instruction per 4-head group.
  - softmax denominator rides along as a ones-column in each head's v tile;
    the division is one scalar_tensor_tensor per output tile using a
    stride-0 broadcast AP.
  - the qkv projection is interleaved with attention per 512-token chunk so
    the PE-heavy projection and the DVE/ACT-heavy softmax overlap.
"""

import os
import numpy as np
from contextlib import ExitStack

import concourse.bacc as bacc
import concourse.bass as bass
import concourse.tile as tile
import concourse.mybir as mybir
from concourse.bass_utils import run_bass_kernel_spmd

BF, T, C = 64, 256, 512
NH, LB, HD = 8, 31, 64
NCORES = 8
BFL = BF // NCORES            # 8 bf rows per core
TOK = BFL * T                 # 2048 tokens per core
NEG = -1.0e30

F32 = mybir.dt.float32
F32R = mybir.dt.float32r
BF16 = mybir.dt.bfloat16
AF = mybir.ActivationFunctionType

# query blocks: (q0, nq, k0, W)  -- window keys [k0, k0+W)
QBLOCKS = [
    (0, 96, 0, 96),
    (96, 96, 64, 128),
    (192, 64, 160, 96),
]
QB_OFF = [0, 768, 1536]       # column offset of each qblock in BT
# output pieces: (qb_idx, col_start_in_ex, nq_piece, psum_base==y_row, y_tile)
PIECES = [
    (0, 0, 96, 0, 0),
    (1, 0, 32, 96, 0),
    (1, 32, 64, 0, 1),
    (2, 0, 64, 64, 1),
]

_CACHE = {}


def _build_module():
    nc = bacc.Bacc("TRN2", target_bir_lowering=False, debug=False,
                   num_devices=NCORES)

    dram = {}
    def din(name, shape, dt):
        dram[name] = nc.dram_tensor(name, shape, dt, kind="ExternalInput").ap()
    din("xT", [C, TOK], BF16)
    din("Wq", [C, 512], BF16)
    din("Wk", [C, 512], BF16)
    din("Wv", [C, 512], BF16)
    din("bqe", [128, 4], F32)
    din("bqo", [128, 4], F32)
    din("maske", [128, 1], F32)
    din("masko", [128, 1], F32)
    din("bk", [128, 4], F32)
    din("bv", [128, 512], BF16)
    din("e0m", [128, 128], BF16)
    din("EB", [128, 2048], BF16)
    din("ones", [1, 128], BF16)
    y_ap = nc.dram_tensor("y", [TOK, C], F32, kind="ExternalOutput").ap()

    NCH = max(1, TOK // 512)
    CW = TOK // NCH

    with tile.TileContext(nc) as tc:
        with ExitStack() as ctx:
            singles = ctx.enter_context(tc.tile_pool(name="singles", bufs=1))

            # ---- persistent SBUF tensors ----
            xT = [singles.tile([128, TOK], BF16, tag=f"xT{i}", name=f"xT{i}")
                  for i in range(4)]
            Wq = [singles.tile([128, 512], BF16, tag=f"Wq{i}", name=f"Wq{i}")
                  for i in range(4)]
            Wk = [singles.tile([128, 512], BF16, tag=f"Wk{i}", name=f"Wk{i}")
                  for i in range(4)]
            Wv = [singles.tile([128, 512], BF16, tag=f"Wv{i}", name=f"Wv{i}")
                  for i in range(4)]
            bqe = singles.tile([128, 4], F32, tag="bqe")
            bqo = singles.tile([128, 4], F32, tag="bqo")
            maske = singles.tile([128, 1], F32, tag="maske")
            masko = singles.tile([128, 1], F32, tag="masko")
            bk = singles.tile([128, 4], F32, tag="bk")
            bv = singles.tile([128, 512], BF16, tag="bv")
            e0m = singles.tile([128, 128], BF16, tag="e0m")
            ones_r = singles.tile([1, 128], BF16, tag="ones_r")
            EB = singles.tile([128, 2048], BF16, tag="EB")
            qTe = [singles.tile([128, TOK], BF16, tag=f"qTe{i}", name=f"qTe{i}")
                   for i in range(4)]
            qTo = [singles.tile([128, TOK], BF16, tag=f"qTo{i}", name=f"qTo{i}")
                   for i in range(4)]
            kT = [singles.tile([128, TOK], BF16, tag=f"kT{i}", name=f"kT{i}")
                  for i in range(4)]

            nc.sync.dma_start(out=ones_r, in_=dram["ones"])

            def load_xt(i, ch):
                nc.sync.dma_start(
                    out=xT[i][:, CW * ch:CW * (ch + 1)],
                    in_=dram["xT"][128 * i:128 * (i + 1), CW * ch:CW * (ch + 1)])
            for i in range(4):
                nc.sync.dma_start(out=Wq[i], in_=dram["Wq"][128 * i:128 * (i + 1), :])
            for i in range(4):
                load_xt(i, 0)
            # small constants next -- they gate psum recycling and the first
            # attention chains
            nc.sync.dma_start(out=bqe, in_=dram["bqe"])
            nc.sync.dma_start(out=bqo, in_=dram["bqo"])
            nc.sync.dma_start(out=maske, in_=dram["maske"])
            nc.sync.dma_start(out=masko, in_=dram["masko"])
            nc.sync.dma_start(out=bk, in_=dram["bk"])
            nc.sync.dma_start(out=bv, in_=dram["bv"])
            nc.sync.dma_start(out=e0m, in_=dram["e0m"])
            nc.sync.dma_start(out=EB, in_=dram["EB"])
            for i in range(4):
                nc.sync.dma_start(out=Wk[i], in_=dram["Wk"][128 * i:128 * (i + 1), :])
            for i in range(4):
                nc.sync.dma_start(out=Wv[i], in_=dram["Wv"][128 * i:128 * (i + 1), :])
            for ch in range(1, NCH):
                for i in range(4):
                    load_xt(i, ch)
            pqkv = ctx.enter_context(
                tc.tile_pool(name="psum_qkv", bufs=2, space="PSUM"))
            pst = ctx.enter_context(
                tc.tile_pool(name="psum_st", bufs=2, space="PSUM"))
            poa = ctx.enter_context(
                tc.tile_pool(name="psum_oa", bufs=2, space="PSUM"))
            vpool = ctx.enter_context(tc.tile_pool(name="vsb", bufs=4))
            epool = ctx.enter_context(tc.tile_pool(name="esb", bufs=8))
            rpool = ctx.enter_context(tc.tile_pool(name="rsb", bufs=6))
            ypool = ctx.enter_context(tc.tile_pool(name="ysb", bufs=6))

            def qk_chunk(ch):
                # q/k projection for token chunk ch -> qTe/qTo/kT columns
                cols = slice(CW * ch, CW * (ch + 1))
                for oct in range(8):       # 0..3: q tiles, 4..7: k tiles
                    Wsb = Wq if oct < 4 else Wk
                    col = 128 * (oct % 4)
                    bias = (bqe if oct < 4 else bk)[:, (oct % 4):(oct % 4) + 1]
                    ps = pqkv.tile([128, 512], F32, tag="qkps", name="qkps")
                    for ic in range(4):
                        nc.tensor.matmul(
                            out=ps[:, 0:CW],
                            lhsT=Wsb[ic][:, col:col + 128],
                            rhs=xT[ic][:, cols],
                            start=(ic == 0), stop=(ic == 3))
                    if oct >= 4:
                        o = kT[oct % 4][:, cols]
                        if oct % 2 == 0:
                            nc.scalar.activation(out=o, in_=ps[:, 0:CW],
                                                 func=AF.Identity, bias=bias)
                        else:
                            nc.vector.tensor_scalar_add(
                                out=o, in0=ps[:, 0:CW], scalar1=bias)
                    else:
                        nc.scalar.activation(
                            out=qTe[oct][:, cols], in_=ps[:, 0:CW],
                            func=AF.Identity, bias=bias, scale=maske)
                        if (oct + ch) % 2 == 0:
                            nc.vector.tensor_scalar(
                                out=qTo[oct][:, cols], in0=ps[:, 0:CW],
                                scalar1=masko,
                                scalar2=bqo[:, (oct % 4):(oct % 4) + 1],
                                op0=mybir.AluOpType.mult, op1=mybir.AluOpType.add)
                        else:
                            nc.scalar.activation(
                                out=qTo[oct][:, cols], in_=ps[:, 0:CW],
                                func=AF.Identity,
                                bias=bqo[:, (oct % 4):(oct % 4) + 1], scale=masko)

            def v_proj(bf, vA, vC):
                t0 = bf * T
                for half, dst in ((0, vA), (1, vC)):
                    ps = pqkv.tile([128, 512], F32, tag="qkps", name="vps")
                    tok0 = t0 + 128 * half
                    for ic in range(4):
                        nc.tensor.matmul(
                            out=ps, lhsT=xT[ic][:, tok0:tok0 + 128],
                            rhs=Wv[ic], start=(ic == 0), stop=False)
                    nc.tensor.matmul(out=ps, lhsT=e0m, rhs=bv,
                                     start=False, stop=True)
                    out3 = dst.rearrange("p (h c) -> p h c", h=NH)[:, :, 0:64]
                    in3 = ps.rearrange("p (h c) -> p h c", h=NH)
                    if (bf + half) % 2 == 0:
                        nc.vector.tensor_copy(out3, in3)
                    else:
                        nc.scalar.activation(out=out3, in_=in3, func=AF.Copy)
                    nc.vector.memset(
                        dst.rearrange("p (h c) -> p h c", h=NH)[:, :, 64:65], 1.0)

            def attention(bf, vwin):
                t0 = bf * T
                # per output row-tile: one 2-bank PSUM accumulator; head
                # group g lives at columns 512g + 65hh
                oab = {}

                def finish(yt):
                    oa = oab[yt]
                    rc = rpool.tile([128, 8], F32, tag="rc", name="rc")
                    yf = ypool.tile([128, C], F32, tag="yf", name="yf")
                    for g in range(2):
                        rin = bass.AP(tensor=oa.tensor,
                                      offset=oa.offset + 512 * g + 64,
                                      ap=[oa.ap[0], [65, 4], [1, 1]])
                        rout = bass.AP(tensor=rc.tensor, offset=rc.offset + 4 * g,
                                       ap=[rc.ap[0], [1, 4], [1, 1]])
                        nc.vector.reciprocal(rout, rin)
                        in0 = bass.AP(tensor=oa.tensor, offset=oa.offset + 512 * g,
                                      ap=[oa.ap[0], [65, 4], [1, 64]])
                        in1 = bass.AP(tensor=rc.tensor, offset=rc.offset + 4 * g,
                                      ap=[rc.ap[0], [1, 4], [0, 64]])
                        out = bass.AP(tensor=yf.tensor, offset=yf.offset + 256 * g,
                                      ap=[yf.ap[0], [64, 4], [1, 64]])
                        nc.vector.scalar_tensor_tensor(
                            out=out, in0=in0, scalar=1.0, in1=in1,
                            op0=mybir.AluOpType.mult, op1=mybir.AluOpType.mult)
                    nc.sync.dma_start(
                        out=y_ap[t0 + 128 * yt:t0 + 128 * (yt + 1), :], in_=yf)

                for qb, (q0, nq, k0, W) in enumerate(QBLOCKS):
                    exh = []
                    fn = 4 * nq
                    for g in range(2):     # head groups 0-3 / 4-7
                        st = pst.tile([128, 512], F32, tag="st", name="st")
                        for hh in range(4):
                            h = 4 * g + hh
                            p = h // 2
                            qs = (qTe if h % 2 == 0 else qTo)[p]
                            nc.tensor.matmul(
                                out=st[0:W, nq * hh:nq * (hh + 1)],
                                lhsT=kT[p][:, t0 + k0:t0 + k0 + W],
                                rhs=qs[:, t0 + q0:t0 + q0 + nq],
                                start=True, stop=True)
                        et = epool.tile([128, 384], BF16, tag="et", name="et")
                        nc.scalar.activation(out=et[0:W, 0:fn], in_=st[0:W, 0:fn],
                                             func=AF.Exp)
                        ex = epool.tile([128, 384], BF16, tag="ex", name="ex")
                        nc.vector.tensor_mul(
                            ex[0:W, 0:fn], et[0:W, 0:fn],
                            EB[0:W, QB_OFF[qb] + fn * g:QB_OFF[qb] + fn * g + fn])
                        exh.append(ex)
                    for (pqb, cs, nqp, b0, yt) in PIECES:
                        if pqb != qb:
                            continue
                        if yt not in oab:
                            oab[yt] = poa.tile([128, 1024], F32, tag="oab",
                                               name="oab")
                        for g in range(2):
                            for hh in range(4):
                                h = 4 * g + hh
                                co = 512 * g + 65 * hh
                                nc.tensor.matmul(
                                    out=oab[yt][b0:b0 + nqp, co:co + 65],
                                    lhsT=exh[g][0:W, nq * hh + cs:nq * hh + cs + nqp],
                                    rhs=vwin[qb][0:W, 65 * h:65 * h + 65],
                                    start=True, stop=True, tile_position=(0, b0))
                    if qb == 1:
                        finish(0)
                finish(1)

            # warm the ACT function table (Exp) during the DMA phase
            dummy = singles.tile([1, 1], F32, tag="dummy")
            nc.scalar.activation(out=dummy, in_=ones_r[0:1, 0:1], func=AF.Exp)


            # stagger: qk chunk ch+1 is emitted after the first bf of chunk
            # ch so the PE has projection work while softmax chains drain
            bf_per_ch = max(1, BFL // NCH)
            order = []
            for ch in range(NCH):
                if ch == 0:
                    order.append(("qk", 0))
                bfs = list(range(bf_per_ch * ch, min(BFL, bf_per_ch * (ch + 1))))
                if bfs:
                    order.append(("att", bfs[0]))
                if ch + 1 < NCH:
                    order.append(("qk", ch + 1))
                for bf in bfs[1:]:
                    order.append(("att", bf))
            for kind, arg in order:
                if kind == "qk":
                    qk_chunk(arg)
                    continue
                bf = arg
                if True:
                    vA = vpool.tile([128, 520], BF16, tag="vA", name="vA")
                    vC = vpool.tile([128, 520], BF16, tag="vC", name="vC")
                    vB = vpool.tile([128, 520], BF16, tag="vB", name="vB")
                    vD = vpool.tile([128, 520], BF16, tag="vD", name="vD")
                    v_proj(bf, vA, vC)
                    nc.sync.dma_start(out=vB[0:64, :], in_=vA[64:128, :])
                    nc.sync.dma_start(out=vB[64:128, :], in_=vC[0:64, :])
                    nc.sync.dma_start(out=vD[0:96, :], in_=vC[32:128, :])
                    attention(bf, {0: vA, 1: vB, 2: vD})

    nc.compile()
    return nc


def _prep_shared(pe, W_qkv, b_qkv):
    r = np.arange(512)
    head = 2 * (r // 128) + (r % 128) // 64
    cc = r % 64
    qsrc = 192 * head + cc
    ksrc = 192 * head + 64 + cc
    j = np.arange(512)
    vsrc = 192 * (j // 64) + 128 + (j % 64)

    import ml_dtypes
    Wq = np.ascontiguousarray((W_qkv[:, qsrc] * 0.125).astype(ml_dtypes.bfloat16))
    Wk = np.ascontiguousarray(W_qkv[:, ksrc].astype(ml_dtypes.bfloat16))
    Wv = np.ascontiguousarray(W_qkv[:, vsrc].astype(ml_dtypes.bfloat16))
    bqv = (b_qkv[qsrc] * 0.125).astype(np.float32).reshape(4, 128).T.copy()
    me = np.zeros((128, 1), np.float32); me[0:64] = 1.0
    mo = np.zeros((128, 1), np.float32); mo[64:128] = 1.0
    bqe_v = bqv * me
    bqo_v = bqv * mo
    bkv = b_qkv[ksrc].astype(np.float32).reshape(4, 128).T.copy()
    bvv = np.zeros((128, 512), dtype=ml_dtypes.bfloat16)
    bvv[0] = b_qkv[vsrc].astype(ml_dtypes.bfloat16)
    e0m = np.zeros((128, 128), dtype=ml_dtypes.bfloat16)
    e0m[0, :] = 1.0

    # BT columns: per qblock, head groups of 4 at 384*g, head-major inside
    BTm = np.full((128, 2048), NEG, dtype=np.float32)
    for qb, (q0, nq, k0, W) in enumerate(QBLOCKS):
        for h in range(NH):
            jj = np.arange(W)[:, None]
            ii = np.arange(nq)[None, :]
            d = (q0 - k0) + ii - jj
            valid = (d >= 0) & (d <= LB)
            idx = np.clip(LB - d, 0, LB)
            blk = np.where(valid, pe[h][idx], NEG).astype(np.float32)
            off = QB_OFF[qb] + 4 * nq * (h // 4) + nq * (h % 4)
            BTm[0:W, off:off + nq] = blk
    EBm = np.exp(np.minimum(BTm, 60.0)).astype(ml_dtypes.bfloat16)
    return dict(Wq=Wq, Wk=Wk, Wv=Wv, bqe=bqe_v, bqo=bqo_v, maske=me,
                masko=mo, bk=bkv, bv=bvv, e0m=e0m, EB=EBm,
                ones=np.ones((1, 128), dtype=ml_dtypes.bfloat16))


def kernel(x, pe, W_qkv, b_qkv, num_heads):
    assert int(num_heads) == NH and x.shape == (BF, T, C)
    if "nc" not in _CACHE:
        _CACHE["nc"] = _build_module()
    nc = _CACHE["nc"]

    shared = _prep_shared(np.asarray(pe, np.float32),
                          np.asarray(W_qkv, np.float32),
                          np.asarray(b_qkv, np.float32))
    in_maps = []
    for c in range(NCORES):
        xs = np.asarray(x[BFL * c:BFL * (c + 1)], np.float32).reshape(TOK, C)
        m = dict(shared)
        import ml_dtypes
        m["xT"] = np.ascontiguousarray(xs.T.astype(ml_dtypes.bfloat16))
        in_maps.append(m)
    res = run_bass_kernel_spmd(nc, in_maps, list(range(NCORES)))
    out = np.stack([res.results[c]["y"].reshape(BFL, T, C)
                    for c in range(NCORES)], axis=0)
    return out.reshape(BF, T, C).astype(np.float32)
